# revision 12
# baseline (speedup 1.0000x reference)
"""Bass/Tile kernel for nn_Attention_9234179687166 on 8 TRN2 NeuronCores.

Reference computation per batch b (B=32, L=K=D=1024):
    q      = query @ W_in.T                    # [L, D]
    scores = q @ context.T                     # [L, K]
    w      = masked_softmax(scores, mask)      # multiplicative mask + renorm
    mix    = w @ context                       # [L, D]
    out    = tanh(concat([mix, q]) @ W_out.T)  # [L, D]

Sharding: data-parallel over batch, 4 batches per core, weights replicated.

All matmul operands are fp16 (PSUM accumulation stays fp32): fp16 matmuls
stream at 1 cycle/row like bf16, the rounding error is 8x smaller than
bf16 (measured end-to-end rel err ~1.8e-3, below the fp32r/bf16 baseline's
2.8e-3), and 2-byte operands ride the DMA xbar transpose engine. Every
transpose (W_in, W_out, query, context, softmax weights) is a
dma_start_transpose — the PE runs nothing but the 640 productive
512-free matmuls per batch.

Transposed operands use packed [P, outer, tile, P] layouts so each xbar
transpose lands in a fully contiguous [P, n*128] destination (the xbar
writes garbage to strided destinations); matmul moving operands then read
strided [[n*128 stride, 4], [1, 128]] access patterns, which stream fine.

Masked softmax (mask m in {0,1}, scores s):
    reference: w0 = softmax(s*m); w = w0*m / (sum(w0*m) + 1e-13)
    Softmax is shift invariant, so with u = (s + 4096)*m (masked -> 0),
    e = exp(u - max(u)) has masked lanes equal to exp(-~4096) == 0, and
    w = e / sum(e) matches the reference up to the 1e-13 term. The
    normalization is a post-exp ACT copy with per-partition scale
    (w = e * (1/sum)), so step 5 is a single 16-matmul accumulation per
    512-wide output chunk followed by tanh straight out of PSUM.

Per (batch, l-half) software pipeline, PE program order:
    A: per l-tile: scores matmuls -> (DVE stt, reduce; ACT exp, scale;
       xbar transpose to wT) off the PE's back
    B: step1 matmuls for the NEXT half (covers A's softmax tails)
    C: step4 matmuls (mix), D: step5 matmuls + tanh + store
Query/context/weight loads, fp16 casts, and xbar transposes are emitted
so the DMA queues run ahead of the PE.
"""

import sys

sys.path.insert(0, "/opt/trn_rl_repo")

import numpy as np

P = 128
D = 1024
TWO_D = 2048
DT = D // P      # 8 tiles over D
CT = TWO_D // P  # 16 tiles over 2D
LARGE = 4096.0
N_CORES = 8
B_FULL = 32
NB = B_FULL // N_CORES  # batches per core

_prog_cache = {}
last_results = None  # BassKernelResults of the most recent kernel() call


def build_program(nb, L, K=1024):
    import concourse.mybir as mybir
    import concourse.tile as tile
    from concourse import bacc

    f32 = mybir.dt.float32
    f16 = mybir.dt.float16
    i32 = mybir.dt.int32
    Alu = mybir.AluOpType
    Act = mybir.ActivationFunctionType
    KT = K // P
    LH = min(512, L)      # l-half width
    LJ = LH // P          # 128-row l tiles per half
    NHALF = L // LH
    NT = nb * NHALF       # total halves

    nc = bacc.Bacc("TRN2", target_bir_lowering=False, debug=False,
                   num_devices=N_CORES)
    q_d = nc.dram_tensor("query", [nb, L, D], f32, kind="ExternalInput")
    c_d = nc.dram_tensor("context", [nb, K, D], f32, kind="ExternalInput")
    m_d = nc.dram_tensor("mask", [nb, L, K], i32, kind="ExternalInput")
    win_d = nc.dram_tensor("W_in", [D, D], f32, kind="ExternalInput")
    wout_d = nc.dram_tensor("W_out", [D, TWO_D], f32, kind="ExternalInput")
    out_d = nc.dram_tensor("out", [nb, L, D], f32, kind="ExternalOutput")

    with tile.TileContext(nc) as tc:
        with (
            tc.tile_pool(name="const", bufs=1) as constp,
            tc.tile_pool(name="wstg", bufs=1) as wstgp,
            tc.tile_pool(name="nat", bufs=4) as natp,
            tc.tile_pool(name="ctx", bufs=2) as ctxp,
            tc.tile_pool(name="ctxT", bufs=1) as ctxTp,
            tc.tile_pool(name="qT", bufs=2) as qTp,
            tc.tile_pool(name="qTr", bufs=2) as qTrp,
            tc.tile_pool(name="mixT", bufs=1) as mixTp,
            tc.tile_pool(name="wT", bufs=1) as wTp,
            tc.tile_pool(name="sm", bufs=2) as smp,
            tc.tile_pool(name="ps_big", bufs=2, space="PSUM") as ps_big,
            tc.tile_pool(name="ps_mm", bufs=4, space="PSUM") as ps_mm,
        ):
            # [p, ei, di, r] = W_in[ei*P + r, di*P + p]  (= W_inT[d, e])
            W_inT = constp.tile([P, DT, DT, P], f16)
            # [p, dj, t, r] = W_out[dj*P + r, t*P + p]   (= W_outT[c, dout])
            W_outT = constp.tile([P, DT, CT, P], f16)

            def emit_win_setup():
                # Weight loads ride the scalar-engine (ACT) hwdge queue so
                # they overlap the sync-queue context prologue; casts on DVE.
                for ei in range(DT):
                    nat = natp.tile([P, D], f32, tag="natc", bufs=3)
                    nc.scalar.dma_start(nat[:], win_d[ei * P:(ei + 1) * P, :])
                    ws = wstgp.tile([P, TWO_D], f16, tag="wstg")
                    nc.vector.tensor_copy(ws[:, :D], nat[:])
                    nc.scalar.dma_start_transpose(W_inT[:, ei], ws[:, :D])

            def emit_wout_setup():
                for dj in range(DT):
                    ws = wstgp.tile([P, TWO_D], f16, tag="wstg")
                    for half in range(2):
                        nat = natp.tile([P, D], f32, tag="natc", bufs=3)
                        nc.scalar.dma_start(
                            nat[:],
                            wout_d[dj * P:(dj + 1) * P, half * D:(half + 1) * D])
                        nc.vector.tensor_copy(
                            ws[:, half * D:(half + 1) * D], nat[:])
                    nc.scalar.dma_start_transpose(W_outT[:, dj], ws[:])

            def emit_ctx_stage(b):
                # ctx_h[p, ki, c] = ctx[ki*P + p, c]; ctxT[p, ki, t, r]:
                # (d = t*P + p, k = ki*P + r). One merged xbar transpose:
                # in col ki*D + di*P + p maps to out tile t = ki*DT + di.
                ctx_h = ctxp.tile([P, KT, D], f16, tag="ctxh")
                ctxT = ctxTp.tile([P, KT, DT, P], f16, tag="ctxT")
                for ki in range(KT):
                    nat = natp.tile([P, D], f32, tag="natc", bufs=3)
                    nc.scalar.dma_start(nat[:], c_d[b, ki * P:(ki + 1) * P, :])
                    nc.vector.tensor_copy(ctx_h[:, ki, :], nat[:])
                nc.scalar.dma_start_transpose(ctxT[:], ctx_h[:])
                return ctx_h, ctxT

            def emit_query_stage(t):
                # qT[p, lj, di, r]: (d = di*P + p, l = l0 + lj*P + r)
                # One merged xbar transpose per half (t = lj*DT + di).
                b, h = divmod(t, NHALF)
                l0 = h * LH
                qT = qTp.tile([P, LJ, DT, P], f16, tag="qT")
                qs = smp.tile([P, LJ, D], f16, tag="qstg", bufs=1)
                for lj in range(LJ):
                    nat = natp.tile([P, D], f32, tag="natq", bufs=2)
                    nc.scalar.dma_start(
                        nat[:], q_d[b, l0 + lj * P: l0 + (lj + 1) * P, :])
                    nc.vector.tensor_copy(qs[:, lj, :], nat[:])
                nc.scalar.dma_start_transpose(qT[:], qs[:])
                return qT

            def emit_step1(qT):
                # qTr[p, ei, l] = q(e = ei*P + p, l)
                qTr = qTrp.tile([P, DT, LH], f16, tag="qTr")
                for ei in range(DT):
                    psq = ps_mm.tile([P, LH], f32, tag="mm")
                    for di in range(DT):
                        nc.tensor.matmul(
                            psq[:], W_inT[:, ei, di, :], qT[:, :, di, :],
                            start=(di == 0), stop=(di == DT - 1))
                    nc.scalar.activation(qTr[:, ei, :], psq[:], Act.Copy)
                return qTr

            def emit_mask_load(b, l0, lj):
                mi = smp.tile([P, K], i32, tag="mask", bufs=2)
                nc.sync.dma_start(
                    mi[:], m_d[b, l0 + lj * P: l0 + (lj + 1) * P, :])
                return mi

            def emit_scores_tile(qTr, ctxT, mi, e_n, lj):
                # scores -> masked softmax -> normalized w into e_n[:, lj]
                pss = ps_big.tile([P, K], f32, tag="big")
                for kh in range(K // 512):
                    for ei in range(DT):
                        nc.tensor.matmul(
                            pss[:, kh * 512:(kh + 1) * 512],
                            qTr[:, ei, lj * P:(lj + 1) * P],
                            ctxT[:, 4 * kh:4 * (kh + 1), ei, :],
                            start=(ei == 0), stop=(ei == DT - 1))
                st = smp.tile([P, 4], f32, tag="stats", bufs=3)
                u_t = smp.tile([P, K], f32, tag="u", bufs=1)
                nc.vector.scalar_tensor_tensor(
                    u_t[:], pss[:], LARGE, mi[:], op0=Alu.add, op1=Alu.mult)
                nc.vector.tensor_reduce(
                    st[:, 0:1], u_t[:], axis=mybir.AxisListType.X,
                    op=Alu.max, negate=True)
                e_sb = smp.tile([P, K], f16, tag="e", bufs=2)
                nc.scalar.activation(
                    e_sb[:], u_t[:], Act.Exp,
                    bias=st[:, 0:1], accum_out=st[:, 1:2])
                rec = smp.tile([P, 1], f32, tag="rec", bufs=3)
                nc.vector.reciprocal(rec[:], st[:, 1:2])
                nc.scalar.activation(
                    e_n[:, lj, :], e_sb[:], Act.Copy, scale=rec[:])

            def emit_step4(ctx_h, wT):
                # mixT[p, di, l] = mix(d = di*P + p, l), normalized
                mixT = mixTp.tile([P, DT, LH], f16, tag="mixT")
                for di in range(DT):
                    psm = ps_mm.tile([P, LH], f32, tag="mm")
                    for ki in range(KT):
                        nc.tensor.matmul(
                            psm[:], ctx_h[:, ki, di * P:(di + 1) * P],
                            wT[:, :, ki, :],
                            start=(ki == 0), stop=(ki == KT - 1))
                    nc.vector.tensor_copy(mixT[:, di, :], psm[:])
                return mixT

            def emit_step5(b, h, mixT, qTr):
                l0 = h * LH
                for lj in range(LJ):
                    pso = ps_big.tile([P, K], f32, tag="big")
                    for dh in range(D // 512):
                        rhs_dj = slice(4 * dh, 4 * (dh + 1))
                        for ci in range(DT):
                            nc.tensor.matmul(
                                pso[:, dh * 512:(dh + 1) * 512],
                                mixT[:, ci, lj * P:(lj + 1) * P],
                                W_outT[:, rhs_dj, ci, :],
                                start=(ci == 0), stop=False)
                        for ci in range(DT):
                            nc.tensor.matmul(
                                pso[:, dh * 512:(dh + 1) * 512],
                                qTr[:, ci, lj * P:(lj + 1) * P],
                                W_outT[:, rhs_dj, DT + ci, :],
                                start=False, stop=(ci == DT - 1))
                        o_sb = smp.tile([P, 512], f32, tag="osb", bufs=2)
                        nc.scalar.activation(
                            o_sb[:], pso[:, dh * 512:(dh + 1) * 512], Act.Tanh)
                        # Stores ride the swdge (gpsimd) queue: their wait on
                        # tanh must not head-of-line-block input loads or
                        # transposes on the hwdge queues.
                        nc.gpsimd.dma_start(
                            out_d[b, l0 + lj * P: l0 + (lj + 1) * P,
                                  dh * 512:(dh + 1) * 512],
                            o_sb[:])

            # ---- prologue ----
            # W_in + first query on the scalar queue, context on sync; W_out
            # (first needed ~55us in, at D(0)) loads after the first query.
            emit_win_setup()
            ctx_tiles = emit_ctx_stage(0)
            qT_cur = emit_query_stage(0)
            emit_wout_setup()
            qTr_cur = emit_step1(qT_cur)
            qT_next = emit_query_stage(1) if NT > 1 else None

            # ---- steady-state over halves ----
            for t in range(NT):
                b, h = divmod(t, NHALF)
                l0 = h * LH
                ctx_h, ctxT = ctx_tiles

                # A: scores + softmax per l-tile (masks prefetched 2 deep);
                # one merged xbar transpose of all four l-tiles' weights
                # (t = lj*KT + ki matches wT's [P, LJ, KT, P] layout).
                wT = wTp.tile([P, LJ, KT, P], f16, tag="wT")
                e_n = smp.tile([P, LJ, K], f16, tag="en", bufs=1)
                masks = [emit_mask_load(b, l0, lj) for lj in range(2)]
                for lj in range(LJ):
                    if lj + 2 < LJ:
                        masks.append(emit_mask_load(b, l0, lj + 2))
                    emit_scores_tile(qTr_cur, ctxT, masks[lj], e_n, lj)
                nc.sync.dma_start_transpose(wT[:], e_n[:])

                # B: step1 for the next half + query prefetch for half t+2
                qTr_next = None
                if t + 1 < NT:
                    qTr_next = emit_step1(qT_next)
                    qT_next = emit_query_stage(t + 2) if t + 2 < NT else None

                # context stage for the next batch (DMA/ACT only)
                if h == NHALF - 1 and b + 1 < nb:
                    next_ctx = emit_ctx_stage(b + 1)
                else:
                    next_ctx = None

                # C: mix, D: output projection + tanh + store
                mixT = emit_step4(ctx_h, wT)
                emit_step5(b, h, mixT, qTr_cur)

                qTr_cur = qTr_next
                if next_ctx is not None:
                    ctx_tiles = next_ctx

    nc.compile()
    return nc


def _get_program(nb, L):
    key = (nb, L)
    if key not in _prog_cache:
        _prog_cache[key] = build_program(nb, L)
    return _prog_cache[key]


def kernel(query, context, mask, W_in, W_out):
    from concourse.bass_utils import run_bass_kernel_spmd

    query = np.ascontiguousarray(query, dtype=np.float32)
    context = np.ascontiguousarray(context, dtype=np.float32)
    W_in = np.ascontiguousarray(W_in, dtype=np.float32)
    W_out = np.ascontiguousarray(W_out, dtype=np.float32)
    B, L, _ = query.shape
    mask3 = np.ascontiguousarray(mask.reshape(B, L, -1), dtype=np.int32)

    nb = B // N_CORES
    nc = _get_program(nb, L)
    in_maps = []
    for c in range(N_CORES):
        b0 = c * nb
        in_maps.append({
            "query": query[b0:b0 + nb],
            "context": context[b0:b0 + nb],
            "mask": mask3[b0:b0 + nb],
            "W_in": W_in,
            "W_out": W_out,
        })
    res = run_bass_kernel_spmd(nc, in_maps, core_ids=list(range(N_CORES)))
    global last_results
    last_results = res
    out = np.concatenate([r["out"] for r in res.results], axis=0)
    return out
